# revision 10
# baseline (speedup 1.0000x reference)
"""Copy-enhanced CodeT5 head (histogram/scatter blend) on 8 TRN2 NeuronCores.

Strategy: data-parallel over (batch, T/2) -> 8 shards of 128 decoder rows.
All large tensors travel as bf16 (host casts), halving HBM traffic vs f32;
the output is written bf16 and upcast on the host.

Per core, for its [128, V] output block:
  A        = sum_h cross_attn[h]            (DVE leaf-pair adds + f32 chain)
  p_gen    = 1/(1+e^-u), u = (A @ (enc@W1))/H + dec.W2 + b
             (e^-u computed on ACT with the EXP table -- no sigmoid table
             swap -- then a DVE reciprocal)
  exp, Z   = exp(logits) streamed bf16, row-sums via ACT accum
  P_copy   = scatter-add of (1-p_gen)/H * (A @ Sel) into a bf16 pair-packed
             accumulator (gpsimd scatter_add). Duplicate source ids are
             pre-combined by the Sel matmul; non-first occurrences go to a
             dump slot. The scatter index row is a pure function of
             input_ids and is built on the HOST; the scatter lib is
             preloaded with a dummy call at t~0.
  out      = exp * (p_gen/Z) + P_copy       (per-tile DVE TS (4x) + TT (2x))

Index metadata (pair ids, parity one-hots, scatter indices) is precomputed
on the host -- O(S) bookkeeping, not tensor compute. The pcopy accumulator
is zeroed 3-way (DVE/GPSIMD/ACT) to keep it off every engine's critical
path. No collectives: every core owns a disjoint output block.
"""
import sys

sys.path.insert(0, "/opt/trn_rl_repo")

import numpy as np
import ml_dtypes

import concourse.bass as bass  # noqa: F401  (registers engine classes)
import concourse.mybir as mybir
from concourse import bacc, bass_utils, library_config
from concourse.tile import TileContext

B, S, T, D, H, V = 4, 512, 256, 1024, 16, 32105
P = 128
NCORES = 8
NPAIR = V // 2 + 2          # 16054 pair slots; pairs 0..16052 hold vocab, 16053 = dump
DUMP = NPAIR - 1
VTILE = 2048
NT = (V + VTILE - 1) // VTILE   # 16 blend tiles, last one 1385 wide
CHUNK = 4096
NCH = (V + CHUNK - 1) // CHUNK  # 8 exp/load chunks, last one 3433 wide

AluOp = mybir.AluOpType
Act = mybir.ActivationFunctionType
f32 = mybir.dt.float32
bf16 = mybir.dt.bfloat16
i16 = mybir.dt.int16

BF = ml_dtypes.bfloat16


def _body(tc, logits_d, enc_d, dec_d, xattn_d, wgw_d, wgb_d,
          pairf_d, cols_d, ident_d, idxs_d, zeros_d, out_d):
    nc = tc.nc
    with tc.tile_pool(name="fix", bufs=1) as fix, \
         tc.tile_pool(name="work", bufs=4) as work, \
         tc.tile_pool(name="opool", bufs=5) as opool, \
         tc.tile_pool(name="psum", bufs=1, space="PSUM") as psum:

        # ---- persistent tiles; pcopy is zeroed 4-way (DVE/gpsimd/ACT/DMA)
        # so no single engine eats the whole 13.7us. The gpsimd scatter
        # library is preloaded explicitly (its ~14us IRAM DMA would
        # otherwise stall the first scatter).
        exp_store = fix.tile([P, V], bf16)
        pcopy = fix.tile([P, NPAIR, 2], bf16)
        nc.gpsimd.load_library(library_config.mlp)
        nc.vector.memset(pcopy[:, 0:1600, :], 0.0)
        nc.gpsimd.memset(pcopy[:, 1600:6054, :], 0.0)
        nc.scalar.memzero(pcopy[:, 6054:11554, :])

        # ---- input DMAs in priority order (sync HWDGE queue) ----
        # xattn first (longest dependency chain), host-transposed contiguous,
        # in two halves so the head-sum reduce can start on the first half
        xh_all = fix.tile([P, S, H], bf16)
        nc.sync.dma_start(out=xh_all[:, 0:S // 2, :], in_=xattn_d[:, 0:S // 2, :])
        nc.sync.dma_start(out=xh_all[:, S // 2:S, :], in_=xattn_d[:, S // 2:S, :])
        zparts = fix.tile([P, NCH], f32)
        # logits chunk 0 lands directly in exp_store (exp runs in place)
        nc.sync.dma_start(out=exp_store[:, 0:CHUNK], in_=logits_d[0])
        enc_all = fix.tile([P, 4, D], bf16)
        nc.sync.dma_start(out=enc_all[:], in_=enc_d)
        nc.sync.dma_start(out=pcopy[:, 11554:NPAIR, :], in_=zeros_d)
        w1b = work.tile([P, D], bf16, tag="wgt", bufs=2)
        nc.sync.dma_start(out=w1b[:], in_=wgw_d[0:1, 0:D].to_broadcast((P, D)))
        w2b = work.tile([P, D], bf16, tag="wgt", bufs=2)
        nc.sync.dma_start(out=w2b[:], in_=wgw_d[0:1, D:2 * D].to_broadcast((P, D)))
        dec_t = work.tile([P, D], bf16, tag="dec", bufs=1)
        nc.sync.dma_start(out=dec_t[:], in_=dec_d[:])
        pair_bc = fix.tile([P, S], f32)
        nc.sync.dma_start(out=pair_bc[:], in_=pairf_d[None, :].to_broadcast((P, S)))
        cols_t = fix.tile([P, 12], f32)
        nc.sync.dma_start(out=cols_t[:], in_=cols_d)
        ident = fix.tile([P, P], f32)
        nc.sync.dma_start(out=ident[:], in_=ident_d)
        wb_bc = fix.tile([P, 1], f32)
        nc.sync.dma_start(out=wb_bc[:], in_=wgb_d[None, :].to_broadcast((P, 1)))
        # host-built scatter indices, pre-replicated for the 8 gpsimd cores
        idxs_all = fix.tile([P, 32], i16)
        nc.sync.dma_start(out=idxs_all[:], in_=idxs_d)
        # remaining logits chunks, straight into exp_store slices
        for k in range(1, NCH):
            off = k * CHUNK
            w_k = min(CHUNK, V - off)
            nc.sync.dma_start(out=exp_store[:, off:off + w_k],
                              in_=logits_d[k][:, 0:w_k])

        # ---- head sum -> A: reduce over the host-interleaved H axis, in
        # two halves so each starts as soon as its xattn half lands ----
        A = fix.tile([P, S], f32)
        nc.vector.tensor_reduce(out=A[:, 0:S // 2], in_=xh_all[:, 0:S // 2, :],
                                axis=mybir.AxisListType.X, op=AluOp.add)
        nc.vector.tensor_reduce(out=A[:, S // 2:S], in_=xh_all[:, S // 2:S, :],
                                axis=mybir.AxisListType.X, op=AluOp.add)

        # ---- A^T via PE transposes ----
        A_T = fix.tile([P, 4, P], f32)
        for kk in range(4):
            tps = psum.tile([P, P], f32, tag="tps", bufs=2, name=f"tps{kk}")
            nc.tensor.transpose(tps[:], A[:, kk * P:(kk + 1) * P], ident[:])
            nc.vector.tensor_copy(out=A_T[:, kk, :], in_=tps[:])
        # fold the source-parity masks into the matmul lhs (per-kk scalars)
        A_TE = fix.tile([P, 4, P], bf16)
        A_TO = fix.tile([P, 4, P], bf16)
        for kk in range(4):
            nc.vector.tensor_scalar(A_TE[:, kk, :], A_T[:, kk, :],
                                    cols_t[:, 4 + kk:5 + kk], None, AluOp.mult)
            nc.vector.tensor_scalar(A_TO[:, kk, :], A_T[:, kk, :],
                                    cols_t[:, 8 + kk:9 + kk], None, AluOp.mult)

        # ---- pair-level combine: comb_l[c,s'] = sum_s A[c,s][pair=][par=l].
        # kk 0/1 first, then the p_gen dot products, then kk 2/3, so the
        # s1 chain and the comb matmuls overlap.
        comb_e = psum.tile([P, S], f32, tag="combe")
        comb_o = psum.tile([P, S], f32, tag="combo")
        sels = []
        def _sel_comb(kk):
            sel = work.tile([P, S], bf16, tag="sel", name=f"sel{kk}", bufs=2)
            nc.vector.tensor_scalar(sel[:], pair_bc[:], cols_t[:, kk:kk + 1],
                                    None, AluOp.is_equal)
            nc.tensor.matmul(comb_e[:], A_TE[:, kk, :], sel[:],
                             start=(kk == 0), stop=(kk == 3),
                             skip_group_check=True)
            nc.tensor.matmul(comb_o[:], A_TO[:, kk, :], sel[:],
                             start=(kk == 0), stop=(kk == 3),
                             skip_group_check=True)
        _sel_comb(0)
        _sel_comb(1)

        # ---- p_gen via the EXP table: x = e^-u, p = 1/(1+x) ----
        u_col = fix.tile([P, 4], f32)
        for kk in range(4):
            junk = work.tile([P, D], bf16, tag="jnk", name=f"junk{kk}", bufs=1)
            nc.vector.scalar_tensor_tensor(out=junk[:], in0=enc_all[:, kk, :],
                                           scalar=1.0, in1=w1b[:],
                                           op0=AluOp.mult, op1=AluOp.mult,
                                           accum_out=u_col[:, kk:kk + 1])
        plin1_ps = psum.tile([P, 1], f32, tag="plin")
        for kk in range(4):
            nc.tensor.matmul(plin1_ps[:], A_T[:, kk, :], u_col[:, kk:kk + 1],
                             start=(kk == 0), stop=(kk == 3),
                             skip_group_check=True)
        _sel_comb(2)
        _sel_comb(3)
        p_lin2 = fix.tile([P, 1], f32)
        junk2 = work.tile([P, D], bf16, tag="jnk", bufs=1)
        nc.vector.scalar_tensor_tensor(out=junk2[:], in0=dec_t[:], scalar=1.0,
                                       in1=w2b[:], op0=AluOp.mult, op1=AluOp.mult,
                                       accum_out=p_lin2[:])
        # negb = -(p_lin2 + wb); wb_bc holds -wgb (host negates)
        negb = fix.tile([P, 1], f32)
        nc.vector.scalar_tensor_tensor(out=negb[:], in0=p_lin2[:], scalar=-1.0,
                                       in1=wb_bc[:], op0=AluOp.mult,
                                       op1=AluOp.add)

        # ---- exp stream on ACT, in place over exp_store; e^-u after chunk 1
        xeu = fix.tile([P, 1], f32)
        emitted_xeu = False
        for k in range(NCH):
            off = k * CHUNK
            w_k = min(CHUNK, V - off)
            nc.scalar.activation(out=exp_store[:, off:off + w_k],
                                 in_=exp_store[:, off:off + w_k],
                                 func=Act.Exp, accum_out=zparts[:, k:k + 1])
            if k == 1 and not emitted_xeu:
                nc.scalar.activation(out=xeu[:], in_=plin1_ps[:], func=Act.Exp,
                                     bias=negb[:], scale=-1.0 / H)
                emitted_xeu = True

        # p_gen = 1/(1+x); s1 = (1-p)/H
        onex = fix.tile([P, 1], f32)
        nc.vector.tensor_scalar(onex[:], xeu[:], 1.0, None, AluOp.add)
        p_gen = fix.tile([P, 1], f32)
        nc.vector.reciprocal(out=p_gen[:], in_=onex[:])
        s1 = fix.tile([P, 1], f32)
        nc.vector.tensor_scalar(s1[:], p_gen[:], -1.0 / H, 1.0 / H,
                                AluOp.mult, AluOp.add)

        # ---- scatter adds (identity layout: add row j = source column j) ----
        add_pairs = fix.tile([P, S, 2], bf16)
        nc.vector.tensor_scalar(add_pairs[:, :, 0], comb_e[:], s1[:],
                                None, AluOp.mult)
        nc.vector.tensor_scalar(add_pairs[:, :, 1], comb_o[:], s1[:],
                                None, AluOp.mult)
        nc.gpsimd.scatter_add(in_ap=pcopy[:], idxs_ap=idxs_all[:],
                              add_ap=add_pairs[:], channels=P, num_elems=NPAIR,
                              d=2, num_idxs=S)

        # ---- softmax scale ----
        Z = fix.tile([P, 1], f32)
        nc.vector.tensor_reduce(out=Z[:], in_=zparts[:], axis=mybir.AxisListType.X,
                                op=AluOp.add)
        invZ = fix.tile([P, 1], f32)
        nc.vector.reciprocal(out=invZ[:], in_=Z[:])
        s0 = fix.tile([P, 1], f32)
        nc.vector.tensor_mul(out=s0[:], in0=p_gen[:], in1=invZ[:])

        # ---- blend: TS (4x bf16) + TT (2x bf16) per tile; DMA out on sync.
        # The first few TS-scales are emitted up front so they overlap the
        # scatter tail (the TT add is what needs pcopy).
        pcopy_flat = pcopy[:].rearrange("p a b -> p (a b)")
        PRE = 4
        otiles = []
        for k in range(NT):
            otiles.append(opool.tile([P, VTILE], bf16, tag="ot", name=f"ot{k}"))

        def _ts(k):
            off = k * VTILE
            w_k = min(VTILE, V - off)
            nc.vector.tensor_scalar(otiles[k][:, :w_k],
                                    exp_store[:, off:off + w_k],
                                    s0[:], None, AluOp.mult)

        for k in range(PRE):
            _ts(k)
        for k in range(NT):
            off = k * VTILE
            w_k = min(VTILE, V - off)
            nc.vector.tensor_add(out=otiles[k][:, :w_k], in0=otiles[k][:, :w_k],
                                 in1=pcopy_flat[:, off:off + w_k])
            nc.sync.dma_start(out=out_d[k][:, :w_k], in_=otiles[k][:, :w_k])
            if k + PRE < NT:
                _ts(k + PRE)


_CACHE = {}


def _get_graph():
    if "nc" in _CACHE:
        return _CACHE["nc"]
    nc = bacc.Bacc("TRN2", target_bir_lowering=False, debug=False,
                   num_devices=NCORES)
    logits_d = nc.dram_tensor("logits", [NCH, P, CHUNK], bf16,
                              kind="ExternalInput").ap()
    enc_d = nc.dram_tensor("enc", [P, 4, D], bf16, kind="ExternalInput").ap()
    dec_d = nc.dram_tensor("dec", [P, D], bf16, kind="ExternalInput").ap()
    xattn_d = nc.dram_tensor("xattn", [P, S, H], bf16, kind="ExternalInput").ap()
    wgw_d = nc.dram_tensor("wgw", [1, 2 * D], bf16, kind="ExternalInput").ap()
    wgb_d = nc.dram_tensor("wgb", [1], f32, kind="ExternalInput").ap()
    pairf_d = nc.dram_tensor("pairf", [S], f32, kind="ExternalInput").ap()
    cols_d = nc.dram_tensor("cols", [P, 12], f32, kind="ExternalInput").ap()
    ident_d = nc.dram_tensor("identf", [P, P], f32, kind="ExternalInput").ap()
    idxs_d = nc.dram_tensor("idxs16", [P, 32], i16, kind="ExternalInput").ap()
    zeros_d = nc.dram_tensor("zeros", [P, NPAIR - 11554, 2], bf16,
                             kind="ExternalInput").ap()
    out_d = nc.dram_tensor("out", [NT, P, VTILE], bf16,
                           kind="ExternalOutput").ap()
    with TileContext(nc) as tc:
        _body(tc, logits_d, enc_d, dec_d, xattn_d, wgw_d, wgb_d,
              pairf_d, cols_d, ident_d, idxs_d, zeros_d, out_d)
    nc.compile()
    _CACHE["nc"] = nc
    return nc


def _retile(block):
    # [P, V] -> [NCH, P, CHUNK] contiguous bf16 chunks (zero-padded tail)
    out = np.zeros((NCH, P, CHUNK), BF)
    for k in range(NCH):
        off = k * CHUNK
        w = min(CHUNK, V - off)
        out[k, :, :w] = block[:, off:off + w]
    return out


def _shard(inputs):
    ids = np.asarray(inputs["input_ids"]).astype(np.int64)
    logits = np.asarray(inputs["logits"], dtype=np.float32)
    enc = np.asarray(inputs["encoder_hidden_states"], dtype=np.float32)
    dec = np.asarray(inputs["decoder_hidden_states"], dtype=np.float32)
    xattn = np.asarray(inputs["cross_attentions"], dtype=np.float32)
    wgw = np.asarray(inputs["W_gen_w"], dtype=np.float32)
    wgb = np.asarray(inputs["W_gen_b"], dtype=np.float32)
    identf = np.eye(P, dtype=np.float32)
    in_maps = []
    for c in range(NCORES):
        b, th = c // 2, c % 2
        t0 = th * P
        ids_b = ids[b]
        pair = (ids_b >> 1).astype(np.float32)
        parity = (ids_b & 1).astype(np.float32)
        cols = np.empty((P, 12), np.float32)
        for kk in range(4):
            seg = slice(kk * P, (kk + 1) * P)
            cols[:, kk] = pair[seg]
            cols[:, 4 + kk] = (parity[seg] == 0.0)
            cols[:, 8 + kk] = (parity[seg] == 1.0)
        # scatter index list: first occurrence of each pair -> slot, else dump
        idx_list = np.full(S, DUMP, np.int16)
        seen = set()
        for j in range(S):
            pr = int(ids_b[j]) >> 1
            if pr not in seen:
                seen.add(pr)
                idx_list[j] = pr
        idxs16 = np.ascontiguousarray(
            np.tile(idx_list.reshape(32, 16).T, (8, 1)))  # [128, 32]
        in_maps.append({
            "logits": _retile(logits[b, t0:t0 + P, :].astype(BF)),
            # enc in column layout [dec-row-partition? no: [p, c, d] chunks]
            "enc": np.ascontiguousarray(
                enc[b].reshape(4, P, D).transpose(1, 0, 2)).astype(BF),
            "dec": np.ascontiguousarray(dec[b, t0:t0 + P, :]).astype(BF),
            # xattn host-transposed to [t-row, s, h] so the DMA is contiguous
            # and the head sum is a single innermost-axis reduce
            "xattn": np.ascontiguousarray(
                xattn[b, :, t0:t0 + P, :].transpose(1, 2, 0)).astype(BF),
            "wgw": wgw.astype(BF),
            "wgb": -wgb,
            "pairf": pair,
            "cols": cols,
            "identf": identf,
            "idxs16": idxs16,
            "zeros": np.zeros((P, NPAIR - 11554, 2), BF),
        })
    return in_maps


def run(inputs, trace=False):
    nc = _get_graph()
    in_maps = _shard(inputs)
    res = bass_utils.run_bass_kernel_spmd(nc, in_maps,
                                          core_ids=list(range(NCORES)),
                                          trace=trace)
    out = np.empty((B, T, V), np.float32)
    for c in range(NCORES):
        b, th = c // 2, c % 2
        tiles = np.asarray(res.results[c]["out"]).astype(np.float32)
        block = np.transpose(tiles, (1, 0, 2)).reshape(P, NT * VTILE)[:, :V]
        out[b, th * P:(th + 1) * P, :] = block
    return out, res


def kernel(**inputs):
    out, _ = run(inputs, trace=False)
    return out
